# revision 21
# baseline (speedup 1.0000x reference)
"""Expert-parallel MoE SwiGLU kernel for Trainium2 (8 NeuronCores).

Strategy: every core processes ALL 8 experts, but only a 512-wide slice of
the intermediate dimension I (core s owns columns [512*s, 512*(s+1)) of
w1/w3 and the matching rows of w2).  Token routing is done host-side:
tokens are gathered per expert and concatenated into one padded column
block per expert.  Each core computes, for every expert e,

    y_e^(s) = (silu(x_e @ w1_e[:, sl]) * (x_e @ w3_e[:, sl])) @ w2_e[sl, :]

a partial output over its I-slice; the host sums the 8 partials and
scatter-adds the weighted per-expert outputs back into the [B, H] result.
This is perfectly load-balanced (all cores run identical token counts) and
avoids padding every expert to the max expert's count.

All matmul operands are bf16 (fp32 PSUM accumulation), halving HBM traffic
so weight streaming stays well below the PE roofline.  Phases are software
pipelined (phase1 of expert e+1 runs between phase1 and phase2 of expert
e) so the silu/mul chain producing h never stalls the PE.  DMAs are
issued one-per-tensor-per-expert (the hardware has only 8 DMA-completion
semaphore lanes; many small DMAs create lane-reuse waits that block the
issuing sequencer and everything queued behind it).
"""

import numpy as np

_P = 128
_E = 8  # experts == cores == I-slices

# (H, I, caps) -> compiled Bass program
_PROG_CACHE = {}
# test hooks: set TRACE=True before calling kernel() to capture an NTFF
# profile; the BassKernelResults of the last run lands in LAST_RUN.
TRACE = False
LAST_RUN = None


def _bf16(a):
    """Fast float32 -> bfloat16 with round-to-nearest-even."""
    import ml_dtypes

    a = np.ascontiguousarray(np.asarray(a, dtype=np.float32))
    u = a.view(np.uint32)
    r = ((u >> 16) & 1) + np.uint32(0x7FFF)
    return ((u + r) >> 16).astype(np.uint16).view(ml_dtypes.bfloat16)


def _chunks(T):
    """Split T into <=512 pieces (8-multiples, as equal as possible)."""
    n = -(-T // 512)
    q, r = divmod(T // 8, n)
    sizes = [(q + (1 if i < r else 0)) * 8 for i in range(n)]
    offs = [sum(sizes[:i]) for i in range(n)]
    return list(zip(offs, sizes))


def _build_program(H, I, caps):
    import concourse.bass as bass
    import concourse.tile as tile
    from concourse import bacc, mybir

    f32 = mybir.dt.float32
    bf16 = mybir.dt.bfloat16
    Silu = mybir.ActivationFunctionType.Silu
    Copy = mybir.ActivationFunctionType.Copy

    HC = H // _P            # 16 h-blocks
    IS = I // _E            # I-slice width per core (512)
    ICL = IS // _P          # 4 i-blocks per core
    offs = [sum(caps[:i]) for i in range(len(caps))]
    TT = sum(caps)

    nc = bacc.Bacc(
        "TRN2",
        target_bir_lowering=False,
        debug=False,
        enable_asserts=False,
        num_devices=_E,
    )
    # per-expert gathered tokens, concatenated: x^T [H, TT] (same on all
    # cores); weights are host-pre-tiled per core so every per-expert DMA
    # moves one large contiguous per-partition segment:
    #   w1/w3: [e][p = h%128][ic][hc][i']   (stationary blocks for phase 1)
    #   w2:    [e][p = i%128][hc2][ic][h']  (stationary blocks for phase 2)
    xT = nc.dram_tensor("xT", [H, TT], bf16, kind="ExternalInput").ap()
    w1 = nc.dram_tensor("w1", [_E, _P, ICL, HC, _P], bf16, kind="ExternalInput").ap()
    w3 = nc.dram_tensor("w3", [_E, _P, ICL, HC, _P], bf16, kind="ExternalInput").ap()
    w2 = nc.dram_tensor("w2", [_E, _P, HC, ICL, _P], bf16, kind="ExternalInput").ap()
    # partial output y^T [H, TT] bf16 (upcast + summed over cores on host)
    y = nc.dram_tensor("y", [H, TT], bf16, kind="ExternalOutput").ap()

    xTr = xT.rearrange("(hc p) t -> p hc t", p=_P)
    yr = y.rearrange("(hc p) t -> p hc t", p=_P)

    with tile.TileContext(nc) as tc:
        with (
            tc.tile_pool(name="xp", bufs=2) as xp,
            tc.tile_pool(name="wp", bufs=2) as wp,
            tc.tile_pool(name="w2p", bufs=2) as w2p,
            tc.tile_pool(name="hp", bufs=3) as hp,
            tc.tile_pool(name="cp", bufs=1) as cp,
            tc.tile_pool(name="sp", bufs=2) as sp,
            tc.tile_pool(name="op", bufs=2) as op,
            tc.tile_pool(name="pp", bufs=4, space="PSUM") as pp,
        ):
            zbias = cp.tile([_P, 1], f32)
            nc.any.memset(zbias[:], 0.0)
            wz = cp.tile([_P, _P], bf16)
            nc.any.memset(wz[:], 0.0)

            xts = {}
            w1ts = {}
            w3ts = {}
            w2ts = {}
            hts = {}
            yts = {}

            def emit_inputs(e, split=False):
                Te = caps[e]
                off = offs[e]
                xt = xp.tile([_P, HC, Te], bf16, tag="x", name=f"x_{e}")
                w1t = wp.tile([_P, ICL, HC, _P], bf16, tag="w1", name=f"w1_{e}")
                w3t = wp.tile([_P, ICL, HC, _P], bf16, tag="w3", name=f"w3_{e}")
                w2t = w2p.tile([_P, HC, ICL, _P], bf16, tag="w2", name=f"w2_{e}")
                if split:
                    # prologue: fine-grained, spread over both HWDGE queues
                    # so the first matmul's deps land as early as possible
                    HCJ = HC // 4
                    nc.sync.dma_start(xt[:, 0:HCJ, :], xTr[:, 0:HCJ, off : off + Te])
                    nc.scalar.dma_start(
                        xt[:, 2 * HCJ : 3 * HCJ, :],
                        xTr[:, 2 * HCJ : 3 * HCJ, off : off + Te],
                    )
                    nc.sync.dma_start(w1t[:, 0], w1[e, :, 0])
                    nc.scalar.dma_start(
                        xt[:, 3 * HCJ :, :], xTr[:, 3 * HCJ :, off : off + Te]
                    )
                    nc.sync.dma_start(
                        xt[:, HCJ : 2 * HCJ, :], xTr[:, HCJ : 2 * HCJ, off : off + Te]
                    )
                    for ic in range(1, ICL):
                        nc.sync.dma_start(w1t[:, ic], w1[e, :, ic])
                    nc.scalar.dma_start(w3t[:, 0:2], w3[e, :, 0:2])
                    nc.scalar.dma_start(w3t[:, 2:], w3[e, :, 2:])
                else:
                    nc.sync.dma_start(xt[:], xTr[:, :, off : off + Te])
                    nc.sync.dma_start(w1t[:], w1[e])
                    nc.scalar.dma_start(w3t[:], w3[e])
                xts[e] = xt
                w1ts[e] = w1t
                w3ts[e] = w3t
                w2ts[e] = w2t
                hts[e] = hp.tile([_P, ICL, Te], bf16, tag="h", name=f"h_{e}")
                yts[e] = op.tile([_P, HC, Te], bf16, tag="yt", name=f"yt_{e}")

            def emit_w2(e):
                nc.sync.dma_start(w2ts[e][:], w2[e])

            def phase1(e):
                Te = caps[e]
                xt, w1t, w3t, hs = xts[e], w1ts[e], w3ts[e], hts[e]
                for ic in range(ICL):
                    for coff, csz in _chunks(Te):
                        pg = pp.tile([_P, 512], f32, tag="ps", bufs=4, name=f"pg_{e}_{ic}")
                        for hc in range(HC):
                            nc.tensor.matmul(
                                pg[:, :csz],
                                lhsT=w1t[:, ic, hc, :],
                                rhs=xt[:, hc, coff : coff + csz],
                                start=(hc == 0),
                                stop=(hc == HC - 1),
                            )
                        pu = pp.tile([_P, 512], f32, tag="ps", bufs=4, name=f"pu_{e}_{ic}")
                        for hc in range(HC):
                            nc.tensor.matmul(
                                pu[:, :csz],
                                lhsT=w3t[:, ic, hc, :],
                                rhs=xt[:, hc, coff : coff + csz],
                                start=(hc == 0),
                                stop=(hc == HC - 1),
                            )
                        sg = sp.tile([_P, 512], f32, tag="sg", name=f"sg_{e}_{ic}")
                        nc.scalar.activation(
                            sg[:, :csz], pg[:, :csz], Silu, bias=zbias[:]
                        )
                        nc.vector.tensor_mul(
                            hs[:, ic, coff : coff + csz], sg[:, :csz], pu[:, :csz]
                        )

            def phase2(e, last=False):
                Te = caps[e]
                off = offs[e]
                w2t, hs, yt = w2ts[e], hts[e], yts[e]
                for hc2 in range(HC):
                    for coff, csz in _chunks(Te):
                        py = pp.tile([_P, 512], f32, tag="ps2", bufs=4, name=f"py_{e}_{hc2}")
                        for ic in range(ICL):
                            nc.tensor.matmul(
                                py[:, :csz],
                                lhsT=w2t[:, hc2, ic, :],
                                rhs=hs[:, ic, coff : coff + csz],
                                start=(ic == 0),
                                stop=(ic == ICL - 1),
                            )
                        # drain PSUM fast: each copy is split across the
                        # vector and scalar engines so py banks recycle at
                        # half-copy latency
                        hsz = (csz * 5 // 8 + 3) & ~3
                        nc.vector.tensor_copy(
                            yt[:, hc2, coff : coff + hsz], py[:, :hsz]
                        )
                        nc.scalar.activation(
                            yt[:, hc2, coff + hsz : coff + csz],
                            py[:, hsz:csz],
                            Copy,
                            bias=0.0,
                        )
                    # flush a quarter of the output rows per 4 hc2 blocks
                    # (one DMA each keeps the total DMA count low); the very
                    # last quarter of the final expert goes per-hc2 so the
                    # tail drains fast
                    if hc2 % 4 == 3 and not (last and hc2 == HC - 1):
                        q0 = hc2 - 3
                        nc.sync.dma_start(
                            yr[:, q0 : hc2 + 1, off : off + Te],
                            yt[:, q0 : hc2 + 1, :],
                        )
                    elif last and hc2 == HC - 1:
                        for h2 in range(hc2 - 3, HC):
                            nc.sync.dma_start(
                                yr[:, h2 : h2 + 1, off : off + Te],
                                yt[:, h2 : h2 + 1, :],
                            )

            emit_inputs(0, split=True)
            emit_inputs(1)
            emit_w2(0)

            # warm the PE p-state while expert-0 inputs stream: dummy
            # matmuls on a zeroed tile, later ones gated on the arriving
            # x / w1 chunks so the busy stretch bridges into the real work
            pdum = pp.tile([_P, 512], f32, tag="ps2", bufs=4, name="pdum")
            HCJ = HC // 4
            for i in range(30):
                nc.tensor.matmul(pdum[:, :_P], lhsT=wz[:], rhs=wz[:])
            for j, hcg in ((0, 0), (2, 2 * HCJ), (1, HCJ), (3, 3 * HCJ)):
                for i in range(3):
                    nc.tensor.matmul(
                        pdum[:, :_P], lhsT=wz[:], rhs=xts[0][:, hcg, 0:_P]
                    )
            for i in range(3):
                nc.tensor.matmul(pdum[:, :_P], lhsT=wz[:], rhs=w1ts[0][:, 0, 0, :])

            for e in range(_E):
                phase1(e)
                if e >= 1:
                    phase2(e - 1)
                if e + 1 < _E:
                    emit_w2(e + 1)
                if e + 2 < _E:
                    emit_inputs(e + 2)
            phase2(_E - 1, last=True)

    nc.compile()
    return nc


def _get_program(H, I, caps):
    key = (H, I, caps)
    if key not in _PROG_CACHE:
        _PROG_CACHE[key] = _build_program(H, I, caps)
    return _PROG_CACHE[key]


def kernel(x, expert_indices, expert_weights, w1, w2, w3):
    global LAST_RUN
    from concourse.bass_utils import run_bass_kernel_spmd

    x = np.ascontiguousarray(np.asarray(x, dtype=np.float32))
    idx = np.asarray(expert_indices).astype(np.int64)
    wts = np.asarray(expert_weights, dtype=np.float32)
    w1 = np.asarray(w1, dtype=np.float32)
    w2 = np.asarray(w2, dtype=np.float32)
    w3 = np.asarray(w3, dtype=np.float32)

    B, H = x.shape
    E, _, I = w1.shape
    assert E == _E, f"expected {_E} experts, got {E}"
    HC = H // _P
    IS = I // _E
    ICL = IS // _P

    # host-side dispatch: per-token expert weight matrix (merges duplicate
    # top-k hits of the same expert), then token lists per expert
    wmat = np.zeros((B, E), np.float32)
    np.add.at(wmat, (np.arange(B)[:, None], idx), wts)
    sel = np.zeros((B, E), bool)
    sel[np.arange(B)[:, None], idx] = True
    toks = [np.nonzero(sel[:, e])[0] for e in range(E)]

    # process experts largest-first (smallest drains last -> shortest tail)
    order = sorted(range(E), key=lambda e: -len(toks[e]))
    caps = tuple(max(16, -(-len(toks[o]) // 8) * 8) for o in order)
    offs = [sum(caps[:i]) for i in range(E)]
    TT = sum(caps)

    nc = _get_program(H, I, caps)

    # gathered, padded x^T [H, TT] in bf16 (identical on every core)
    xb = _bf16(x)
    xcat = np.zeros((H, TT), xb.dtype)
    for i, o in enumerate(order):
        te = toks[o]
        xcat[:, offs[i] : offs[i] + len(te)] = xb[te].T

    # pre-tile the weights for all cores at once (bf16, contiguous DMA):
    #  w1/w3: [s, e, p=h%128, ic, hc, i'] ; w2: [s, e, p=i%128, hc2, ic, h']
    po = np.asarray(order)
    w1b = _bf16(w1).reshape(E, HC, _P, _E, ICL, _P).transpose(3, 0, 2, 4, 1, 5)
    w1b = np.ascontiguousarray(w1b[:, po])
    w3b = _bf16(w3).reshape(E, HC, _P, _E, ICL, _P).transpose(3, 0, 2, 4, 1, 5)
    w3b = np.ascontiguousarray(w3b[:, po])
    w2b = _bf16(w2).reshape(E, _E, ICL, _P, HC, _P).transpose(1, 0, 3, 4, 2, 5)
    w2b = np.ascontiguousarray(w2b[:, po])

    in_maps = [
        {"xT": xcat, "w1": w1b[s], "w3": w3b[s], "w2": w2b[s]}
        for s in range(_E)
    ]
    res = run_bass_kernel_spmd(nc, in_maps, list(range(_E)), trace=TRACE)
    LAST_RUN = res

    ysum = res.results[0]["y"].astype(np.float32)
    for s in range(1, _E):
        ysum += res.results[s]["y"].astype(np.float32)

    out = np.zeros((B, H), np.float32)
    for i, o in enumerate(order):
        te = toks[o]
        if len(te):
            out[te] += wmat[te, o][:, None] * ysum[:, offs[i] : offs[i] + len(te)].T
    return out
